# revision 3
# baseline (speedup 1.0000x reference)
"""Bilateral blur (7x7, L1 color distance) on 8 Trainium2 NeuronCores.

Input (4, 3, 512, 512) fp32 -> output (4, 3, 512, 512) fp32.

Sharding: pure data parallelism - core i handles batch i//2, row-half i%2
(256 output rows x 512 cols). The host reflect-pads each image and re-lays
each shard into a "transposed-blocked" layout: partition p (0..127) owns
output columns [4p, 4p+4); its free dim holds, per channel, a 10x262 patch
(padded cols 4p..4p+9 x 262 padded rows, flattened xl*262+y). TRN2 compute
engines cannot read across partitions, so this layout turns all 49 tap
shifts (dy, dx) into pure free-dim AP offsets (dx*262 + dy).

Per tap: dlt = taps-center (fat fp16 TT over 3 channels), |dlt| (ACT Abs),
d = ch-sum (2 TT), q = d^2 (ACT Square), w = exp(-50q + ln s_k) (ACT Exp,
scale/bias immediates), products w*tap (fp16 TT), and a single fat fp32
accumulate of [m0|m1|m2|w]. Final out = num * reciprocal(den).

fp16 notes: all hot DVE ops are TensorTensor (2x DVE mode in
fp16; the TensorScalarPtr family is 1x-only on this ISA so it is avoided).
-50 rides the exp's scale immediate; ln(s_k) rides its per-partition bias AP
(from a small DMA'd table). Taps are read from one of two host-provided fp16
copies (second shifted one row) so every tap AP is 4-byte aligned (dy parity),
which the DVE 2x mode requires.
"""
import numpy as np

import concourse.bass as bass
import concourse.bacc as bacc
import concourse.mybir as mybir
from concourse.tile import TileContext
from concourse import bass_utils

C = 3
B, H, W = 4, 512, 512
KX = KY = 7
PAD = 3
SIGMA_COLOR = 0.1
N_CORES = 8

ROWS = 256
WG = 4
NPART = 128
XE, YE = WG + 2 * PAD, ROWS + 2 * PAD
FREE_IN = XE * YE
FREE_OUT = WG * ROWS
NTAP = KY * KX


def _space_kernel():
    def g1(k, sigma):
        x = np.arange(k, dtype=np.float64) - (k - 1) / 2.0
        g = np.exp(-0.5 * (x / sigma) ** 2)
        return g / g.sum()
    gy, gx = g1(KY, 1.5), g1(KX, 1.5)
    return (gy[:, None] * gx[None, :]).reshape(-1)


def _tap_ap(t, dx, dy, nch=C, ch0=0, dy_base=0):
    a = t[:]
    return bass.AP(a.tensor, a.offset + ch0 * FREE_IN + dx * YE + (dy - dy_base),
                   [[C * FREE_IN, NPART], [FREE_IN, nch], [YE, WG], [1, ROWS]])


def _stk_ap(t, nch=C, ch0=0, step0=False):
    a = t[:]
    tot = a.shape[1]
    return bass.AP(a.tensor, a.offset + ch0 * FREE_OUT,
                   [[tot, NPART], [0 if step0 else FREE_OUT, nch], [ROWS, WG], [1, ROWS]])


def _build(ntaps=NTAP):
    nc = bacc.Bacc()
    f32 = mybir.dt.float32
    f16 = mybir.dt.float16
    xe = nc.dram_tensor("xe", [NPART, C * FREE_IN], f16, kind="ExternalInput")
    xo = nc.dram_tensor("xo", [NPART, C * FREE_IN], f16, kind="ExternalInput")
    lnsb = nc.dram_tensor("lnsb", [NPART, NTAP], f32, kind="ExternalInput")
    o = nc.dram_tensor("o", [NPART, C * FREE_OUT], f32, kind="ExternalOutput")
    AOT = mybir.AluOpType
    F = FREE_OUT
    SCL = -0.5 / SIGMA_COLOR ** 2

    with TileContext(nc) as tc:
        with tc.tile_pool(name="persist", bufs=1) as pool, \
             tc.tile_pool(name="tmp", bufs=3) as tp:
            Te = pool.tile([NPART, C * FREE_IN], f16, name="Te")
            nc.sync.dma_start(Te[:], xe[:])
            To = pool.tile([NPART, C * FREE_IN], f16, name="To")
            nc.sync.dma_start(To[:], xo[:])
            bias = pool.tile([NPART, NTAP], f32, name="bias")
            nc.sync.dma_start(bias[:], lnsb[:])

            def tile_for(dy):
                return (Te, 0) if dy % 2 == 0 else (To, 1)

            # fp32 accumulator [m0|m1|m2|w]: the dominant fp16 error source is
            # accumulator rounding (measured 2.8e-3 -> 5.3e-4 with fp32).
            # One contiguous tile makes the per-tap fp32 accumulation a single
            # fat add.
            acc = pool.tile([NPART, 4 * F], f32, name="acc")
            nc.gpsimd.memset(acc[:], 0.0)
            qb = pool.tile([NPART, F], f16, name="qb")

            tc_t, tc_b = tile_for(PAD)
            ctr = _tap_ap(tc_t, PAD, PAD, dy_base=tc_b)
            for dy in range(KY):
                for dx in range(KX):
                    k = dy * KX + dx
                    if k >= ntaps:
                        continue
                    tt, tb = tile_for(dy)
                    dlt = tp.tile([NPART, C * F], f16, name="dlt", tag="dlt")
                    nc.vector.tensor_tensor(out=_stk_ap(dlt),
                                            in0=_tap_ap(tt, dx, dy, dy_base=tb),
                                            in1=ctr, op=AOT.subtract)
                    adl = tp.tile([NPART, C * F], f16, name="adl", tag="adl")
                    nc.scalar.activation(adl[:], dlt[:],
                                         mybir.ActivationFunctionType.Abs,
                                         bias=0.0, scale=1.0)
                    dsum = tp.tile([NPART, F], f16, name="dsum", tag="dsum")
                    nc.vector.tensor_tensor(out=dsum[:], in0=adl[:, 0:F],
                                            in1=adl[:, F:2 * F], op=AOT.add)
                    nc.vector.tensor_tensor(out=dsum[:], in0=dsum[:],
                                            in1=adl[:, 2 * F:], op=AOT.add)
                    # q = d^2 on ACT (same table set as Exp/Abs -> no set switch);
                    # keeps the DVE critical path shorter
                    nc.scalar.activation(qb[:], dsum[:],
                                         mybir.ActivationFunctionType.Square,
                                         bias=0.0, scale=1.0)
                    mAll = tp.tile([NPART, 4 * F], f16, name="mAll", tag="mAll")
                    # w = exp(-50*q + ln s_k) lands in mAll's 4th slot
                    nc.scalar.activation(mAll[:, 3 * F:], qb[:],
                                         mybir.ActivationFunctionType.Exp,
                                         bias=bias[:, k:k + 1], scale=SCL)
                    wv = mAll[:, 3 * F:]
                    w_b2 = bass.AP(wv.tensor, wv.offset,
                                   [[4 * F, NPART], [0, 2], [ROWS, WG], [1, ROWS]])
                    nc.vector.tensor_tensor(out=_stk_ap(mAll, nch=2),
                                            in0=w_b2,
                                            in1=_tap_ap(tt, dx, dy, nch=2, dy_base=tb),
                                            op=AOT.mult)
                    nc.vector.tensor_tensor(out=mAll[:, 2 * F:3 * F],
                                            in0=mAll[:, 3 * F:],
                                            in1=_tap_ap(tt, dx, dy, nch=1, ch0=2,
                                                        dy_base=tb),
                                            op=AOT.mult)
                    nc.vector.tensor_tensor(out=acc[:], in0=acc[:], in1=mAll[:],
                                            op=AOT.add)

            recip = pool.tile([NPART, F], f32, name="recip")
            nc.vector.reciprocal(recip[:], acc[:, 3 * F:])
            ot = pool.tile([NPART, C * F], f32, name="ot")
            nc.vector.tensor_tensor(out=_stk_ap(ot), in0=acc[:, 0:3 * F],
                                    in1=_stk_ap(recip, step0=True), op=AOT.mult)
            nc.sync.dma_start(o[:], ot[:])
    return nc


_COLIDX = np.arange(NPART)[:, None] * WG + np.arange(XE)[None, :]


def _shard_layout(shard, yshift):
    buf = np.zeros((NPART, C, XE, YE), np.float16)
    for c in range(C):
        blk = shard[c].T[_COLIDX]
        if yshift:
            buf[:, c, :, :YE - yshift] = blk[:, :, yshift:]
        else:
            buf[:, c] = blk
    return buf.reshape(NPART, C * FREE_IN)


_LNSB = np.broadcast_to(
    np.log(_space_kernel()).astype(np.float32)[None, :], (NPART, NTAP)).copy()

_NC_CACHE = {}


def _get_nc():
    if "nc" not in _NC_CACHE:
        nc = _build()
        nc.finalize()
        _NC_CACHE["nc"] = nc
    return _NC_CACHE["nc"]


def make_in_maps(x):
    xp = np.pad(x, ((0, 0), (0, 0), (PAD, PAD), (PAD, PAD)), mode="reflect")
    in_maps = []
    for core in range(N_CORES):
        b, half = core // 2, core % 2
        r0 = half * ROWS
        shard = xp[b, :, r0:r0 + ROWS + 2 * PAD, :]
        in_maps.append({"xe": _shard_layout(shard, 0),
                        "xo": _shard_layout(shard, 1),
                        "lnsb": _LNSB})
    return in_maps


def kernel(input: np.ndarray) -> np.ndarray:
    x = np.asarray(input, dtype=np.float32)
    assert x.shape == (B, C, H, W)
    in_maps = make_in_maps(x)
    nc = _get_nc()
    res = bass_utils.run_bass_kernel_spmd(nc, in_maps, list(range(N_CORES)))
    out = np.empty((B, C, H, W), np.float32)
    for core in range(N_CORES):
        b, half = core // 2, core % 2
        r0 = half * ROWS
        ov = np.asarray(res.results[core]["o"]).reshape(NPART, C, WG, ROWS)
        for c in range(C):
            out[b, c, r0:r0 + ROWS, :] = ov[:, c].transpose(2, 0, 1).reshape(ROWS, W)
    return out
